# revision 35
# baseline (speedup 1.0000x reference)
"""Trainium2 Bass kernel for nn_Attention_72404558676364.

Math: the reference computes
    pre[l,b,:] = hs_encoder[l,b,:] @ We.T + (hidden @ Wh.T + b_att)[b,:]
    attn[b,l]  = pre[l,b,:] . v
    out        = softmax(attn, axis=l)
Softmax over l is shift-invariant, so the hidden/Wh/b_att term (constant in
l for fixed b) cancels exactly and the einsum collapses to a single matvec:
    attn[b,l] = hs_encoder[l,b,:] . w_eff,   w_eff = We.T @ v

Precision: hs_encoder and We are shipped to the device as fp16 (halves HBM
traffic, which is the binding resource: the two NeuronCores of an HBM pair
sustain ~350 GB/s each when both stream); all PE accumulation is fp32 in
PSUM.  Measured end-to-end output error vs the fp32 reference is ~1.8e-3
(softmax is dominated by its top-1 weight, so common-mode score error
cancels).

Sharding: data-parallel over batch; core c handles batches [8c, 8c+8).
hs shards are pre-transposed/cast on the host to a batch-major layout
[p=128, j, hc, l] so each batch j is ONE contiguous 1 MiB DMA piece holding
all 8 contraction chunks.  A batch's 8 accumulating matmuls therefore run
as soon as its own piece lands, and its softmax overlaps later batches'
DMA — nothing stacks at the end except the final batch's short chain.
"""

import sys

import numpy as np

for _p in (
    "/root/.axon_site",
    "/root/.axon_site/_ro/trn_rl_repo",
    "/root/.axon_site/_ro/pypackages",
):
    if _p not in sys.path:
        sys.path.append(_p)

import concourse.bass as bass
import concourse.mybir as mybir
import concourse.tile as tile
from concourse.bass_utils import run_bass_kernel_spmd

H = 1024
L = 512
B = 64
NCORES = 8
BC = B // NCORES  # batches per core
P = 128
HC = H // P  # 128-row chunks of the contraction dim

F32 = mybir.dt.float32
F16 = mybir.dt.float16

_split_n = 0


def _split_multi_waits(nc):
    """Hoist extra sem waits onto same-engine NOPs.

    The walrus build in this container rejects any instruction carrying more
    than one sync-wait ("Too many sync wait commands"), but Tile emits
    multi-wait instructions whenever one op depends on several producers.
    A NOP on the same engine immediately before the instruction waits
    equivalently (per-engine program order).
    """
    global _split_n
    engines = [
        mybir.EngineType.SP,
        mybir.EngineType.Activation,
        mybir.EngineType.DVE,
        mybir.EngineType.PE,
        mybir.EngineType.Pool,
    ]
    for fn in nc.m.functions:
        for blk in fn.blocks:
            new_insts = []
            for inst in blk.instructions:
                si = getattr(inst, "sync_info", None)
                if si is not None and si.on_wait and len(si.on_wait) > 1:
                    waits = list(si.on_wait)
                    si.on_wait = waits[:1]
                    # The exit drain carries one wait per DMA queue sem; its
                    # waits may run on ANY engine because the all-engine
                    # barrier right after it orders everything.  Mid-kernel
                    # instructions need same-engine NOPs (program order).
                    wide = (
                        isinstance(inst, mybir.InstDrain) and len(waits) > 3
                    )
                    for k, w in enumerate(waits[1:]):
                        _split_n += 1
                        eng = engines[k % len(engines)] if wide else inst.engine
                        new_insts.append(
                            mybir.InstNoOp(
                                name=f"I-wsplit-{_split_n}",
                                engine=eng,
                                sync_info=mybir.SyncInfo(
                                    on_wait=[w], on_update=[]
                                ),
                                bass_nofuse=True,
                            )
                        )
                new_insts.append(inst)
            blk.instructions = new_insts


def _build():
    nc = bass.Bass(target_bir_lowering=False, enable_partition_id=False)
    # hsp[p, j*HC*L + hc*L + l] = hs[l, 8c+j, hc*128+p], fp16
    hsp = nc.dram_tensor("hsp", [P, BC * HC * L], F16, kind="ExternalInput")
    # wed[p, hc*H + k] = We[hc*128+p, k], fp16
    wed = nc.dram_tensor("We", [P, HC * H], F16, kind="ExternalInput")
    # vd[p, hc] = v[hc*128+p], fp16
    vd = nc.dram_tensor("v", [P, HC], F16, kind="ExternalInput")
    # 1x1 identity for the PE transposes (engines cannot memset at
    # partition bases other than 0/32/64/96, so it ships from the host)
    identd = nc.dram_tensor("ident8", [8, 8], F32, kind="ExternalInput")
    out = nc.dram_tensor("out", [BC, L], F32, kind="ExternalOutput")
    # 4-byte sink for the junk-matmul reader chain
    dbg = nc.dram_tensor("dbg", [1, 1], F32, kind="ExternalOutput")

    with tile.TileContext(nc) as tc:
        with (
            tc.tile_pool(name="singles", bufs=1) as singles,
            tc.tile_pool(name="psw", bufs=1, space="PSUM") as psw_pool,
            tc.tile_pool(name="pst", bufs=1, space="PSUM") as pst_pool,
            tc.tile_pool(name="pss", bufs=2, space="PSUM") as pss_pool,
            tc.tile_pool(name="psj", bufs=1, space="PSUM") as psj_pool,
        ):
            # ---- input DMAs, all queued up front on the sync HWDGE ring.
            # FIFO order on one ring = exactly the arrival order the
            # pipeline wants; each transfer stripes across all 16 SDMA
            # engines, so one ring already achieves line rate.
            v_sb = singles.tile([P, HC], F16)
            nc.sync.dma_start(out=v_sb[:], in_=vd[:])
            ident8 = singles.tile([8, 8], F32)
            nc.sync.dma_start(out=ident8[:], in_=identd[:])
            we_sb = []
            for wh in range(2):  # chunks 0-3, then 4-7
                t = singles.tile([P, 4 * H], F16, name=f"we{wh}")
                nc.sync.dma_start(
                    out=t[:], in_=wed[:, wh * 4 * H : (wh + 1) * 4 * H]
                )
                we_sb.append(t)
            hs_sb = []
            for j in range(BC):
                t = singles.tile([P, HC * L], F16, name=f"hs{j}")
                # Later pieces ride the scalar ring: it is idle until the
                # first out DMA (~36 us), and splitting the queue tail
                # across both HWDGE rings lets descriptor processing for
                # the final pieces proceed in parallel.
                eng = nc.sync if j < 5 else nc.scalar
                eng.dma_start(
                    out=t[:], in_=hsp[:, j * HC * L : (j + 1) * HC * L]
                )
                hs_sb.append((t,))

            # ---- HAM warmup: the PE clock sits gated at 1.2 GHz until it
            # has been busy ~3.4 us.  Junk matmuls (zeroed fp16 operands,
            # own PSUM bank) start the moment the engine-init barrier
            # drops, so every real matmul below runs at 2.4 GHz.
            jw = singles.tile([P, 1], F16)
            nc.vector.memset(jw[:], 0.0)
            jr = singles.tile([P, L], F16)
            nc.vector.memset(jr[:], 0.0)
            jp = psj_pool.tile([1, L], F32)

            def junk(n, cols=L):
                for _ in range(n):
                    nc.tensor.matmul(
                        jp[0:1, 0:cols], lhsT=jw[:, 0:1], rhs=jr[:, 0:cols],
                        start=True, stop=True,
                    )

            junk(8)

            # ---- w_row = We.T @ v as [1, H] fp32: v chunk is the
            # stationary [128, 1] operand, We chunk streams; the two
            # k-halves run on PE column-groups 0 and 1, accumulating into
            # rows 0 / 32 of one PSUM bank.
            ph = psw_pool.tile([P, L], F32)
            for hc in range(HC):
                for half in range(2):
                    nc.tensor.matmul(
                        ph[32 * half : 32 * half + 1, :],
                        lhsT=v_sb[:, hc : hc + 1],
                        rhs=we_sb[hc // 4][
                            :, (hc % 4) * H + half * L : (hc % 4) * H + half * L + L
                        ],
                        start=(hc == 0),
                        stop=(hc == HC - 1),
                        tile_position=(0, 32 * half),
                    )

            # ---- w_cols[p, hc] = w_eff[hc*128+p], fp16.  Engine SBUF
            # accesses must start at quadrant-aligned partitions, so the
            # row is first staged to SBUF (two aligned copies), then each
            # 128-slice is PE-transposed into one column of a PSUM tile;
            # a single cast-copy produces the fp16 column tile.
            w_row = singles.tile([1, H], F32)
            for half in range(2):
                nc.scalar.copy(
                    out=w_row[0:1, half * L : (half + 1) * L],
                    in_=ph[32 * half : 32 * half + 1, :],
                )
            pt = pst_pool.tile([P, HC], F32)
            for hc in range(HC):
                nc.tensor.transpose(
                    pt[:, hc : hc + 1],
                    w_row[0:1, hc * P : (hc + 1) * P],
                    ident8[0:1, 0:1],
                )
            w_cols = singles.tile([P, HC], F16)
            nc.scalar.copy(out=w_cols[:], in_=pt[:])
            junk(2, cols=256)

            # ---- scores + softmax, batch-major.  Batch j accumulates its
            # 8 fp16 matmuls into PSUM row 32*(j%4) of its group's bank
            # (tile_position col-groups), right behind its own DMA piece.
            # The whole softmax then runs partition-parallel directly on
            # the [128, L] PSUM bank — no gather copies.  Unwritten PSUM
            # rows compute junk that nothing reads.
            for g in range(2):
                ps = pss_pool.tile([P, L], F32, name=f"ps{g}")
                for r in range(4):
                    j = 4 * g + r
                    pieces = hs_sb[j]
                    for hc in range(HC):
                        if len(pieces) == 1:
                            rhs = pieces[0][:, hc * L : (hc + 1) * L]
                        else:
                            rhs = pieces[hc // 4][
                                :, (hc % 4) * L : (hc % 4 + 1) * L
                            ]
                        nc.tensor.matmul(
                            ps[32 * r : 32 * r + 1, :],
                            lhsT=w_cols[:, hc : hc + 1],
                            rhs=rhs,
                            start=(hc == 0),
                            stop=(hc == HC - 1),
                            tile_position=(0, 32 * r),
                        )
                    # keep the PE activity monitor busy through the short
                    # DMA-semaphore wait before the next batch's piece
                    if j < BC - 1:
                        junk(2, cols=256)
                    if j == BC - 2:
                        # Terminal reader for the junk PSUM bank (tile
                        # release check); placed here so the 4-byte DMA
                        # overlaps the final batch instead of the drain.
                        scrap = singles.tile([1, 1], F32)
                        nc.scalar.copy(out=scrap[:], in_=jp[0:1, 0:1])
                        nc.gpsimd.dma_start(out=dbg[0:1, :], in_=scrap[:])
                negmax = singles.tile([P, 1], F32, name=f"nm{g}")
                nc.vector.reduce_max(
                    out=negmax[:], in_=ps[:],
                    axis=mybir.AxisListType.X, negate=True,
                )
                exps = singles.tile([P, L], F32, name=f"ex{g}")
                sums = singles.tile([P, 1], F32, name=f"sm{g}")
                nc.scalar.activation(
                    out=exps[:],
                    in_=ps[:],
                    func=mybir.ActivationFunctionType.Exp,
                    bias=negmax[:],
                    scale=1.0,
                    accum_out=sums[:],
                )
                rsum = singles.tile([P, 1], F32, name=f"rs{g}")
                nc.vector.reciprocal(out=rsum[:], in_=sums[:])
                orow = singles.tile([P, L], F32, name=f"or{g}")
                # Split the normalize into halves, each followed by its own
                # strided-partition out DMA on a different HWDGE ring: the
                # first half's descriptor gen and HBM write overlap the
                # second half's multiply, and the two receipts overlap.
                for h in range(2):
                    sl = slice(h * (L // 2), (h + 1) * (L // 2))
                    nc.vector.tensor_scalar_mul(
                        out=orow[:, sl], in0=exps[:, sl], scalar1=rsum[:]
                    )
                    eng = nc.scalar if h == 0 else nc.sync
                    eng.dma_start(
                        out=out[4 * g : 4 * g + 4, sl],
                        in_=orow[0:97:32, sl],
                    )

    _split_multi_waits(nc)
    return nc


_NC_CACHE = None


def _make_in_maps(hs_encoder, W_att, vector):
    We = np.asarray(W_att)[:, H:].astype(np.float16)  # [H, H]
    we_dev = np.ascontiguousarray(
        We.reshape(HC, P, H).transpose(1, 0, 2).reshape(P, HC * H)
    )
    v_np = np.ascontiguousarray(
        np.asarray(vector, dtype=np.float32)[:, 0]
        .astype(np.float16)
        .reshape(HC, P)
        .T
    )
    hs16 = np.asarray(hs_encoder).astype(np.float16)  # [L, B, H]

    in_maps = []
    for c in range(NCORES):
        sh = hs16[:, c * BC : (c + 1) * BC, :]  # [L, BC, H]
        t = sh.transpose(2, 1, 0).reshape(HC, P, BC, L)  # [hc, p, j, l]
        t = np.ascontiguousarray(
            t.transpose(1, 2, 0, 3).reshape(P, BC * HC * L)
        )  # [p, j, hc, l]
        in_maps.append(
            {"hsp": t, "We": we_dev, "v": v_np,
             "ident8": np.eye(8, dtype=np.float32)}
        )
    return in_maps


def kernel(hidden, hs_encoder, W_att, b_att, vector):
    global _NC_CACHE
    if _NC_CACHE is None:
        _NC_CACHE = _build()
    nc = _NC_CACHE

    in_maps = _make_in_maps(hs_encoder, W_att, vector)
    res = run_bass_kernel_spmd(nc, in_maps, core_ids=list(range(NCORES)))
    out = np.concatenate(
        [_extract_out(res.results[c]["out"]) for c in range(NCORES)], axis=0
    )
    return out[:, None, :].astype(np.float32)


def _extract_out(dev):
    return np.asarray(dev).reshape(BC, L)
